# revision 44
# baseline (speedup 1.0000x reference)
"""Neural ODE (explicit Euler, 20 steps) Trainium2 Bass kernel — fp8 DoubleRow.

z_{s+1} = z_s + h * (tanh(z_s @ W1 + b1) @ W2 + b2),  z0: [8192, 512] f32.

Pure data parallel over 8 NeuronCores (1024 batch rows each); state kept
feature-major (zT: [512 features, 1024 batch]) resident in SBUF for all 20
steps. Matmuls run in fp8 e4m3 with perf_mode=DoubleRow (K=256 per matmul,
2 fp8 weights per PE cell), halving the matmul count vs fp16: per step per
core 32 DoubleRow MMs (mm1 + mm2, each 2 chunks x 4 output tiles x 2 K-pairs).

Accuracy plan (numpy-simulated rel_max ~8e-3 vs 2e-2 budget):
 - fp32 master state zm, updated from fp32 PSUM each step (no fp8 error
   accumulates in the state); fresh fp8 quantization of z per step.
 - weights quantized with *error feedback* over NCOPIES=5 cycling copies:
   copy_i = q8(W - sum_of_previous_copy_errors), so partial sums of the
   systematic weight-quantization error stay bounded (~1 quantization step)
   instead of growing linearly over 20 steps.
 - power-of-2 scaling keeps everything in e4m3 normal range: W1 grid = W1*2^9
   (descaled by the tanh's scale=2^-9), W2 grid = W2*(h*2^12) (descaled by
   the state update's fused scalar 2^-12, which also applies h).

Engine split per step (measured 8.4us/step steady state):
PE 32 MMs at warm cadence ~215ns (full fp8 double-pump rate; LDWEIGHTS
hidden; jl-outer order matters — bank-alternating accumulation order
measured +44ns/MM). All PSUM tiles are 2-bank [128,2,512]; ph and py come
from SEPARATE 2-buf pools (ph-bank reuse gates on tanh, py-bank reuse on
the next step's mm2 — much laxer than a shared ring), and mm2_c0 is
interleaved into the mm1 stream so the DVE queue starts ~0.9us earlier.
Per bank-pair x: chunk 0 runs the serial form (DVE zm_x = zm + 2^-12*py_x,
py_x's ONLY reader so PSUM frees after one op; ACT re-quantizes
zr_x = fp8(zm_x) from SBUF); chunk 1 runs the parallel form (DVE zr_x then
zm_x straight from PSUM). Measured loads: DVE 6x1213ns = 7.3us, ACT
6x1115ns = 6.7us, PE 6.9us + ~1.0us of ACT-chain stalls. PE idle gaps stay
under the ~3.4us HAM MID window so the clock holds 2.4 GHz for the whole
kernel (303.8us fp16 baseline -> 188.5us).

The period-setting loop is PE -> tanh_c1_23 -> zr_c0_01 -> zr_c0_23 -> PE
(three serial ACT links). Measured dead ends for breaking it: splitting a
zr tile across two writer engines (ACT+DVE or ACT+GPSIMD halves) LOSES
3-6us — Tile's semaphore consolidation around multi-writer tiles adds
serialization that outweighs the shorter chain. Keep one writer per tile.
GPSIMD CAST: 3.6us/FD1024 (unusable on any chain); a persistent 8-bank
PSUM tile (sub-tile WAR deps) doubled PE stalls vs the split pools.

Hot path assumes b1 = b2 = 0 (true for this problem's inputs); a general
with_bias build is compiled lazily only if nonzero biases ever show up.
"""

import numpy as np
import ml_dtypes

P = 128
D = 512
B_FULL = 8192
NCORES = 8
BSH = B_FULL // NCORES  # 1024 batch rows per core
NSTEPS = 20
FT = D // P             # 4 feature tiles
CB = 512                # batch columns per chunk
NCHUNK = BSH // CB      # 2 chunks
NWARM = 4               # data-independent PE prewarm matmuls (HAM clock ramp).
                        # With the z16 seed DMA split/reordered by need, the
                        # earlier step-0 start no longer stalls on z16 (the
                        # failure mode that made NWARM<8 regress before)
NCOPIES = 5             # error-feedback fp8 weight copies (cycled over steps)
SW1 = 512.0             # W1 fp8 grid scale (descaled via tanh scale)
SW2 = 4096.0            # W2 fp8 grid carries h*SW2; descaled by C2 in DVE
C1 = 1.0 / SW1
C2 = 1.0 / SW2

_CACHE = {}


def _build_nc(with_bias):
    import concourse.bacc as bacc
    import concourse.mybir as mybir
    import concourse.tile as tile

    f32 = mybir.dt.float32
    f16 = mybir.dt.float16
    f8 = mybir.dt.float8e4
    Tanh = mybir.ActivationFunctionType.Tanh
    Copy = mybir.ActivationFunctionType.Copy
    DR = mybir.MatmulPerfMode.DoubleRow
    MUL = mybir.AluOpType.mult
    ADD = mybir.AluOpType.add

    nc = bacc.Bacc("TRN2", target_bir_lowering=False, debug=False)
    # z transposed on host: [D, BSH] feature-major
    z8_in = nc.dram_tensor("z8", [D, BSH], f8, kind="ExternalInput")
    z16_in = nc.dram_tensor("z16", [D, BSH], f16, kind="ExternalInput")
    w1_in = [
        nc.dram_tensor(f"w1_{i}", [P, FT, D], f8, kind="ExternalInput")
        for i in range(NCOPIES)
    ]
    w2_in = [
        nc.dram_tensor(f"w2_{i}", [P, FT, D], f8, kind="ExternalInput")
        for i in range(NCOPIES)
    ]
    if with_bias:
        # biases[p, jt, s] = b1[jt*128+p] + s * (W1^T (h*b2))[jt*128+p]
        b_in = nc.dram_tensor("biases", [P, FT, NSTEPS], f32, kind="ExternalInput")
        # bfin[p, jt] = NSTEPS * h * b2[jt*128+p] / C2
        bf_in = nc.dram_tensor("bfin", [P, FT], f32, kind="ExternalInput")
    z_out = nc.dram_tensor("zout", [D, BSH], f32, kind="ExternalOutput")

    z8_t = z8_in.ap().rearrange("(ft p) b -> p ft b", p=P)
    z16_t = z16_in.ap().rearrange("(ft p) b -> p ft b", p=P)
    zout_t = z_out.ap().rearrange("(ft p) b -> p ft b", p=P)

    def cslice(c):
        return slice(c * CB, (c + 1) * CB)

    with tile.TileContext(nc) as tc:
        with (
            tc.tile_pool(name="wpool", bufs=1) as wpool,
            tc.tile_pool(name="zpool", bufs=2) as zpool,
            tc.tile_pool(name="zrpool", bufs=2) as zrpool,
            tc.tile_pool(name="apool", bufs=2) as apool,
            tc.tile_pool(name="php", bufs=2, space="PSUM") as php,
            tc.tile_pool(name="pyp", bufs=2, space="PSUM") as pyp,
        ):
            # ---- PE prewarm with real fp8 DoubleRow matmuls (transpose-mode
            # does NOT count as PE-busy for the HAM clock monitor, so warm
            # with the same instruction type the steps use; ramps the clock
            # to 2.4 GHz while the input DMAs run) ----
            wtile = wpool.tile([P, 2, CB], f8, tag="wtile")
            nc.vector.memset(wtile[:], 1.0)
            warm_sink = wpool.tile([P, P], f32, tag="warm")
            # preload the tanh ACT table set while DMAs run
            nc.scalar.activation(
                warm_sink[0:1, 0:1], wtile[0:1, 0, 0:1], Tanh,
            )
            # warm in the py pool: its slots aren't needed until ~2.6us into
            # step 0, so step 0's first mm1 grab (ph pool) is never gated by
            # warm-MM WAW chains (measured ~1.5us gap when warming in ph)
            for i in range(NWARM):
                wps = pyp.tile([P, 2, CB], f32, tag="py", name=f"warm{i}")
                nc.tensor.matmul(
                    wps[:, 0, :], wtile[:, :, 0:P], wtile[:],
                    start=True, stop=True, perf_mode=DR,
                )
                if i == 0:
                    # consume the first warm tile only: a reader on the LAST
                    # warm MM's psum was measured to delay step 0 by ~0.9us
                    # (its copy gated the slot step 0's first matmul needs)
                    nc.vector.tensor_copy(warm_sink[:], wps[:, 0, 0:P])

            # ---- input DMAs, ordered by first use ----
            z8sb = wpool.tile([P, FT, BSH], f8, tag="z8")
            nc.sync.dma_start(z8sb[:, :, cslice(0)], z8_t[:, :, cslice(0)])
            w1sb = [
                wpool.tile([P, FT, D], f8, tag=f"w1_{i}", name=f"w1_{i}")
                for i in range(NCOPIES)
            ]
            w2sb = [
                wpool.tile([P, FT, D], f8, tag=f"w2_{i}", name=f"w2_{i}")
                for i in range(NCOPIES)
            ]
            nc.sync.dma_start(w1sb[0][:], w1_in[0].ap())
            # z16 (fp16 master seed) split into quarter-slices and
            # interleaved by first-use time: step 0's DVE state ops need
            # z16_c0 halves ~2.6us after step 0 starts — a single 1MB z16
            # DMA queued later lands at ~14us and gates the whole chain
            # (this was why earlier step-0 starts regressed)
            z16sb = wpool.tile([P, FT, BSH], f16, tag="z16")
            nc.sync.dma_start(
                z16sb[:, 0:2, cslice(0)], z16_t[:, 0:2, cslice(0)])
            nc.sync.dma_start(z8sb[:, :, cslice(1)], z8_t[:, :, cslice(1)])
            nc.sync.dma_start(w2sb[0][:], w2_in[0].ap())
            nc.sync.dma_start(
                z16sb[:, 2:4, cslice(0)], z16_t[:, 2:4, cslice(0)])
            nc.sync.dma_start(
                z16sb[:, 0:2, cslice(1)], z16_t[:, 0:2, cslice(1)])
            nc.sync.dma_start(
                z16sb[:, 2:4, cslice(1)], z16_t[:, 2:4, cslice(1)])
            if with_bias:
                bias_sb = wpool.tile([P, FT, NSTEPS], f32, tag="bias")
                nc.sync.dma_start(bias_sb[:], b_in.ap())
                bfin_sb = wpool.tile([P, FT], f32, tag="bfin")
                nc.sync.dma_start(bfin_sb[:], bf_in.ap())
            for i in range(1, NCOPIES):
                nc.sync.dma_start(w1sb[i][:], w1_in[i].ap())
                nc.sync.dma_start(w2sb[i][:], w2_in[i].ap())

            # state kept as 2-bank-pair granules: [chunk][half] where half h
            # covers feature tiles {2h, 2h+1} (= K-pair h for matmul rhs)
            def hslice(h):
                return slice(2 * h, 2 * h + 2)

            zr_cur = [[z8sb[:, hslice(h), cslice(c)] for h in range(2)]
                      for c in range(NCHUNK)]
            zm_cur = [[z16sb[:, hslice(h), cslice(c)] for h in range(2)]
                      for c in range(NCHUNK)]

            # ---- 20 Euler steps ----
            # PE phase order interleaves mm2_c0 into the mm1 stream:
            #   mm1_c0h0, mm1_c0h1, mm1_c1h0, mm2_c0h0, mm1_c1h1,
            #   mm2_c0h1, mm2_c1h0, mm2_c1h1
            # so py_c0h0 lands ~0.9us earlier and the DVE queue starts
            # sooner; ph and py use separate 2-buf pools so py-bank reuse
            # is gated by the NEXT step's mm2 (lots of slack) instead of
            # its mm1.
            for s in range(NSTEPS):
                wi = s % NCOPIES
                last = s == NSTEPS - 1
                a_t = {}

                def emit_mm1(c, h):
                    ph = php.tile([P, 2, CB], f32, tag="ph",
                                  name=f"ph{s}_{c}_{h}")
                    # jl-outer: same-bank accumulate pairs run back-to-back
                    # (bank-alternating order measured +44ns/MM on the PE)
                    for jl in range(2):
                        jt = 2 * h + jl
                        for kp in range(2):
                            nc.tensor.matmul(
                                ph[:, jl, :],
                                w1sb[wi][:, hslice(kp), jt * P:(jt + 1) * P],
                                zr_cur[c][kp],
                                start=(kp == 0), stop=(kp == 1),
                                perf_mode=DR,
                            )
                    a = apool.tile([P, 2, CB], f8, tag=f"a{c}_{h}",
                                   name=f"a{s}_{c}_{h}")
                    if with_bias:
                        for jl in range(2):
                            nc.scalar.activation(
                                a[:, jl, :], ph[:, jl, :], Tanh,
                                bias=bias_sb[:, 2 * h + jl, s:s + 1],
                                scale=C1,
                            )
                    else:
                        nc.scalar.activation(a[:], ph[:], Tanh, scale=C1)
                    a_t[(c, h)] = a

                def emit_mm2(c, h):
                    py = pyp.tile([P, 2, CB], f32, tag="py",
                                  name=f"py{s}_{c}_{h}")
                    for jl in range(2):
                        jt2 = 2 * h + jl
                        for kp in range(2):
                            nc.tensor.matmul(
                                py[:, jl, :],
                                w2sb[wi][:, hslice(kp), jt2 * P:(jt2 + 1) * P],
                                a_t[(c, kp)][:],
                                start=(kp == 0), stop=(kp == 1),
                                perf_mode=DR,
                            )
                    zm_new = zpool.tile([P, 2, CB], f32, tag=f"z{c}_{h}",
                                        name=f"zm{s}_{c}_{h}")
                    if not last:
                        zr_new = zrpool.tile([P, 2, CB], f8, tag=f"zr{c}_{h}",
                                             name=f"zr{s}_{c}_{h}")
                        if c == 0:
                            # serial form: zm is py's only reader (PSUM
                            # frees after one op); ACT re-quantizes from
                            # SBUF (GPSIMD CAST measured 3.6us — unusable)
                            nc.vector.scalar_tensor_tensor(
                                zm_new[:], py[:], C2, zm_cur[c][h],
                                MUL, ADD,
                            )
                            nc.scalar.activation(
                                zr_new[:], zm_new[:], Copy, scale=1.0,
                            )
                        else:
                            # parallel form: zr reads PSUM directly (on the
                            # PE critical path); zm deferred until after
                            # BOTH halves' zr (safe now: with split pools
                            # the py-bank reuse deadline is the next step's
                            # mm2, not its mm1)
                            nc.vector.scalar_tensor_tensor(
                                zr_new[:], py[:], C2, zm_cur[c][h],
                                MUL, ADD,
                            )
                            zm_deferred.append((py, zm_new, zm_cur[c][h]))
                        zr_cur[c][h] = zr_new[:]
                        zm_cur[c][h] = zm_new[:]
                    else:
                        if with_bias:
                            tmp = zpool.tile([P, 2, CB], f32,
                                             tag=f"tmp{c}_{h}",
                                             name=f"tmp{s}_{c}_{h}")
                            for jl in range(2):
                                jt2 = 2 * h + jl
                                nc.vector.tensor_scalar(
                                    tmp[:, jl, :], py[:, jl, :],
                                    C2, bfin_sb[:, jt2:jt2 + 1], MUL, ADD,
                                )
                            nc.vector.tensor_add(
                                zm_new[:], tmp[:], zm_cur[c][h])
                        else:
                            nc.vector.scalar_tensor_tensor(
                                zm_new[:], py[:], C2, zm_cur[c][h],
                                MUL, ADD,
                            )
                        nc.sync.dma_start(
                            zout_t[:, hslice(h), cslice(c)], zm_new[:])

                zm_deferred = []
                emit_mm1(0, 0)
                emit_mm1(0, 1)
                emit_mm1(1, 0)
                emit_mm2(0, 0)
                emit_mm1(1, 1)
                emit_mm2(0, 1)
                emit_mm2(1, 0)
                emit_mm2(1, 1)
                for py, zm_new, zm_old in zm_deferred:
                    nc.vector.scalar_tensor_tensor(
                        zm_new[:], py[:], C2, zm_old, MUL, ADD,
                    )

    nc.finalize()
    return nc


def _get_nc(with_bias):
    key = ("nc", with_bias)
    if key not in _CACHE:
        _CACHE[key] = _build_nc(with_bias)
    return _CACHE[key]


def _q8(x):
    """fp32/64 -> TRN e4m3 (max +-240) with RNE, as ml_dtypes.float8_e4m3."""
    return np.clip(np.asarray(x, dtype=np.float32), -240.0, 240.0).astype(
        ml_dtypes.float8_e4m3
    )


def _feedback_copies(W, scale, n):
    """n fp8 copies of W*scale with error feedback: partial sums of the
    per-copy quantization errors stay bounded by ~one quantization step."""
    Wd = W.astype(np.float64) * scale
    cum = np.zeros_like(Wd)
    out = []
    for _ in range(n):
        q = _q8(Wd - cum)
        out.append(q)
        cum = cum + (q.astype(np.float64) - Wd)
    return out


def _tile_w(q):
    # [D, D] (k, j) -> [P, FT, D] with k = kt*128 + p
    return np.ascontiguousarray(q.reshape(FT, P, D).transpose(1, 0, 2))


def _prepare_inputs(z0, t, W1, b1, W2, b2):
    z0 = np.asarray(z0, dtype=np.float32)
    t = np.asarray(t, dtype=np.float64)
    W1 = np.asarray(W1, dtype=np.float64)
    b1 = np.asarray(b1, dtype=np.float64)
    W2 = np.asarray(W2, dtype=np.float64)
    b2 = np.asarray(b2, dtype=np.float64)

    h = (float(t[1]) - float(t[0])) / NSTEPS
    with_bias = bool(np.any(b1 != 0.0) or np.any(b2 != 0.0))

    zT = np.ascontiguousarray(z0.T)                    # [D, B_FULL] f32
    zT16 = zT.astype(np.float16)
    zT8 = _q8(zT)

    w1c = [_tile_w(q) for q in _feedback_copies(W1, SW1, NCOPIES)]
    w2c = [_tile_w(q) for q in _feedback_copies(W2, h * SW2, NCOPIES)]

    shared = {}
    for i in range(NCOPIES):
        shared[f"w1_{i}"] = w1c[i]
        shared[f"w2_{i}"] = w2c[i]
    if with_bias:
        b2h = b2 * h
        wtb = W1.T @ b2h  # [D]
        biases = np.stack(
            [b1 + s * wtb for s in range(NSTEPS)], axis=0
        ).astype(np.float32)  # [NSTEPS, D]
        shared["biases"] = np.ascontiguousarray(
            biases.reshape(NSTEPS, FT, P).transpose(2, 1, 0)
        )
        shared["bfin"] = np.ascontiguousarray(
            (NSTEPS * b2h / C2).astype(np.float32).reshape(FT, P).T
        )

    in_maps = []
    for i in range(NCORES):
        m = {
            "z8": np.ascontiguousarray(zT8[:, i * BSH:(i + 1) * BSH]),
            "z16": np.ascontiguousarray(zT16[:, i * BSH:(i + 1) * BSH]),
        }
        m.update(shared)
        in_maps.append(m)
    return in_maps


def _run(in_maps, trace=False):
    from concourse import bass_utils

    nc = _get_nc("biases" in in_maps[0])
    res = bass_utils.run_bass_kernel_spmd(
        nc, in_maps, core_ids=list(range(NCORES)), trace=trace,
    )
    return res


def kernel(z0, t, W1, b1, W2, b2):
    in_maps = _prepare_inputs(z0, t, W1, b1, W2, b2)
    res = _run(in_maps)
    outT = np.concatenate([r["zout"] for r in res.results], axis=1)  # [D, B]
    return np.ascontiguousarray(outT.T).astype(np.float32)
